# revision 42
# baseline (speedup 1.0000x reference)
"""DigitCaps dynamic-routing kernel for 8 Trainium2 NeuronCores.

Problem: u_hat = einsum('rkoi,bri->brko', W[0], x); 3 routing iterations of
softmax-over-R / weighted-sum / squash / batch-mean agreement.
B=128, R=4608, K=2, O=32, I=16.

Strategy: shard R across the 8 cores (576 routes each).  u_hat (151 MB) is
NEVER materialized -- every routing contraction is pushed through the
factors x and W:

  s~[b,ko]  = sum_{r,i} x[b,r,i] * (exp(b_ij) ⊙ W)[r,i,ko]   (PE, psum-accum)
  a[r,k]    = sum_{i,o} Wt[r,i,ko] * G[r,i,ko],
  G[r,i,ko] = sum_b x[b,r,i] * v[b,ko]                        (PE)

Cross-core combination is an AllReduce(add) of the [128, 66] payload
(s~ partial + Z broadcast into all rows), so no local rank-reduction is
needed after the collective.  A dummy 8-byte collective issued at the very
start of the program absorbs NRT's first-collective barrier + ring setup
behind the input-load phase.  b_ij updates are purely local to the r-shard;
the final squash runs on the host from the per-core partials.
"""

import sys

sys.path.insert(0, "/opt/trn_rl_repo")

import numpy as np

# Problem shapes (hardcoded; harness contract)
B, R, I, K, O = 128, 4608, 16, 2, 32
KO = K * O  # 64
NCORES = 8
RLOC = R // NCORES  # 576 routes per core
NG = RLOC // 8  # 72 groups of 8 routes (8r x 16i = 128 partitions)
NW = NG // 8  # 9 waves of 8 groups
NUM_ITER = 3

_PROGRAM = None  # cached nc


def _build_program(n_iter=NUM_ITER, enable_d=True):
    import concourse.bass as bass
    import concourse.tile as tile
    from concourse import bacc, mybir

    f32 = mybir.dt.float32
    bf16 = mybir.dt.bfloat16
    AF = mybir.ActivationFunctionType
    ALU = mybir.AluOpType

    nc = bacc.Bacc(
        "TRN2",
        target_bir_lowering=False,
        debug=False,
        num_devices=NCORES,
    )

    groups = [list(range(NCORES))]

    # ---------------- I/O ----------------
    xn_d = nc.dram_tensor("xn", [B, RLOC * I], bf16, kind="ExternalInput")
    # xt in partition-major layout: [p=(j,i), (g, b)] -- contiguous per partition
    xt_d = nc.dram_tensor("xt", [128, NG * B], bf16, kind="ExternalInput")
    wt_d = nc.dram_tensor("wt", [128, NG * KO], bf16, kind="ExternalInput")
    e88_d = nc.dram_tensor("e88", [128, 128], bf16, kind="ExternalInput")
    id_d = nc.dram_tensor("id128", [128, 128], bf16, kind="ExternalInput")
    # final iteration outputs the PARTIAL s~ + local Z; the host sums the 8
    # partials and applies the last squash (part of the unshard step)
    vout_d = nc.dram_tensor("v_out", [B, KO + 2], f32, kind="ExternalOutput")

    PAY = KO + 2  # 66
    # bf16 payload: collective latency is what matters, halve the bytes
    cc_in = [
        nc.dram_tensor(f"cc_in{t}", [B, PAY], bf16, kind="Internal")
        for t in range(NUM_ITER)
    ]
    cc_out = [
        nc.dram_tensor(
            f"cc_out{t}", [NCORES, B, PAY], bf16, kind="Internal", addr_space="Shared"
        )
        for t in range(NUM_ITER)
    ]

    with tile.TileContext(nc) as tc:
        with (
            tc.tile_pool(name="persist", bufs=1) as persist,
            tc.tile_pool(name="work", bufs=3) as work,
            tc.tile_pool(name="stats", bufs=4) as stats,
            tc.tile_pool(name="psum_s", bufs=2, space="PSUM") as psum_s,
            tc.tile_pool(name="psum_g", bufs=3, space="PSUM") as psum_g,
            tc.tile_pool(name="psum_m", bufs=3, space="PSUM") as psum_m,
        ):
            # ------------- persistent SBUF state -------------
            xn_s = persist.tile([128, RLOC * I], bf16)  # [b | (g,j,i)]
            xt_s = persist.tile([128, NG, B], bf16)  # [(j,i) | g, b]
            wt_s = persist.tile([128, NG * KO], bf16)  # [(j,i) | (g,k,o)]
            cwt_s = persist.tile([128, NG * KO], bf16)  # c-scaled W
            e88_s = persist.tile([128, 128], bf16)  # kron(eye8, ones16x16)
            ones128 = persist.tile([128, 128], bf16)
            # b_ij, i-duplicated, bf16 SBUF copy: [16j+i | (w, jj, k)].
            # Updated per wave via PE: b_new = e88^T a1w + Id b_old (two
            # matmuls into a fresh psum tile), then copied back by scalar.
            b_sb = persist.tile([128, 2 * NG], bf16)
            id_s = persist.tile([128, 128], bf16)
            zi0 = persist.tile([128, K], f32)
            i32 = mybir.dt.int32
            magic = persist.tile([128, K], i32)
            nc.vector.memset(magic[:], 0x5F3759DF)

            # ------------- load inputs (sync HWDGE ring, priority order) ----
            for c in range(2):
                f0, f1 = 2304 * c, 2304 * (c + 1)
                nc.sync.dma_start(out=wt_s[:, f0:f1], in_=wt_d.ap()[:, f0:f1])
            for c in range(6):
                g0, g1 = 12 * c, 12 * (c + 1)
                nc.sync.dma_start(
                    out=xt_s[:, g0:g1, :], in_=xt_d.ap()[:, 128 * g0 : 128 * g1]
                )
            for c in range(4):
                f0, f1 = 2304 * c, 2304 * (c + 1)
                nc.sync.dma_start(out=xn_s[:, f0:f1], in_=xn_d.ap()[:, f0:f1])
            nc.sync.dma_start(out=e88_s[:], in_=e88_d.ap())
            nc.sync.dma_start(out=id_s[:], in_=id_d.ap())
            # constants built on-chip (vector engine is idle here)
            nc.vector.memset(b_sb[:], 0.0)
            nc.vector.memset(zi0[:], 1.0 / R)
            nc.vector.memset(ones128[:], 1.0)

            # it0 (b)-pass: c uniform -> use wt directly (1/R folded in zi0)
            st_ps = psum_s.tile([128, KO], f32, tag="stilde")
            for g in range(NG):
                nc.tensor.matmul(
                    st_ps[:],
                    lhsT=xt_s[:, g, :],
                    rhs=wt_s[:, KO * g : KO * (g + 1)],
                    start=(g == 0),
                    stop=(g == NG - 1),
                )

            for it in range(n_iter):
                final = it == n_iter - 1
                # ------------- payload: s~ partial + Z in all rows -------------
                pdt = f32 if final else bf16
                payload = work.tile([128, PAY], pdt, tag=f"payload{int(final)}")
                nc.scalar.activation(
                    out=payload[:, 0:KO], in_=st_ps[:], func=AF.Copy
                )
                if it == 0:
                    nc.vector.memset(
                        payload[:, KO : KO + 2],
                        float(R) / NCORES if final else 0.0,
                    )
                else:
                    # local Z_k = sum_r exp(b) = (sum_p sum_{w,jj} exp(b))/16,
                    # broadcast to all 128 rows via ones128 matmul
                    e_all = stats.tile([128, 2 * NG], f32, tag="e_all")
                    nc.scalar.activation(out=e_all[:], in_=b_sb[:], func=AF.Exp)
                    zpart = stats.tile([128, K], f32, tag="zpart")
                    nc.vector.tensor_reduce(
                        out=zpart[:],
                        in_=e_all[:].rearrange(
                            "p (w jj k) -> p k (w jj)", w=NW, jj=8, k=K
                        ),
                        axis=mybir.AxisListType.X,
                        op=ALU.add,
                    )
                    zpart_bf = stats.tile([128, K], bf16, tag="zpartbf")
                    nc.vector.tensor_copy(out=zpart_bf[:], in_=zpart[:])
                    zps = psum_m.tile([128, 16], f32, tag="b2")
                    nc.tensor.matmul(
                        zps[:, 0:K],
                        lhsT=ones128[:],
                        rhs=zpart_bf[:],
                        start=True,
                        stop=True,
                    )
                    nc.scalar.activation(
                        out=payload[:, KO : KO + 2],
                        in_=zps[:, 0:K],
                        func=AF.Copy,
                        scale=1.0 / 16.0,
                    )

                if final:
                    # final iteration: ship partial s~ + local Z to the host
                    nc.sync.dma_start(out=vout_d.ap(), in_=payload[:])
                    continue

                # payload out on the SCALAR ring (sync ring may still be
                # draining xn), then AllGather across the 8 cores
                nc.scalar.dma_start(out=cc_in[it].ap(), in_=payload[:])
                nc.gpsimd.collective_compute(
                    "AllGather",
                    ALU.bypass,
                    replica_groups=groups,
                    ins=[cc_in[it].ap()],
                    outs=[cc_out[it].ap()],
                )
                # readback split across both HWDGE rings (sub-512B segments
                # make this DMA latency-bound; halve it)
                sall = work.tile([128, NCORES, PAY], bf16, tag="sall")
                H = NCORES // 2
                for half, eng in ((0, nc.sync), (1, nc.scalar)):
                    src = bass.AP(
                        tensor=cc_out[it],
                        offset=half * H * PAY * B,
                        ap=[[PAY, 128], [PAY * B, H], [1, PAY]],
                    )
                    eng.dma_start(out=sall[:, half * H : (half + 1) * H, :], in_=src)
                sred = work.tile([128, PAY], f32, tag="sred")
                shalf = work.tile([128, PAY], f32, tag="shalf")
                nc.vector.tensor_reduce(
                    out=shalf[:],
                    in_=sall[:, 0:H, :].rearrange("b r f -> b f r"),
                    axis=mybir.AxisListType.X,
                    op=ALU.add,
                )
                nc.vector.tensor_reduce(
                    out=sred[:],
                    in_=sall[:, H:NCORES, :].rearrange("b r f -> b f r"),
                    axis=mybir.AxisListType.X,
                    op=ALU.add,
                )
                nc.vector.tensor_add(sred[:], sred[:], shalf[:])
                ssum = sred[:, 0:KO]
                if it == 0:
                    zi = zi0
                else:
                    zi = stats.tile([128, K], f32, tag="zi")
                    nc.vector.reciprocal(out=zi[:], in_=sred[:, KO : KO + 2])

                # ------------- squash -------------
                # sn = (sum_o s~^2) * zi^2 ; v = s~ * zi * sqrt(sn)/(0.5+sn)
                ssq = stats.tile([128, K], f32, tag="ssq")
                s2 = work.tile([128, KO], f32, tag="s2")
                nc.vector.tensor_mul(s2[:], ssum, ssum)
                nc.vector.tensor_reduce(
                    out=ssq[:],
                    in_=s2[:].rearrange("p (k o) -> p k o", k=K),
                    axis=mybir.AxisListType.X,
                    op=ALU.add,
                )
                # rsqrt(ssq) via bit-trick seed + 1 Newton step -- runs on
                # ssq directly so it does NOT wait for the zi path; zi^2 is
                # folded in at the end:  sqrt(sn)/(0.5+sn)*zi
                #   = sqrt(ssq)*zi^2 / (0.5 + ssq*zi^2)
                ybits = stats.tile([128, K], i32, tag="ybits")
                nc.vector.tensor_scalar(
                    ybits[:], ssq[:].bitcast(i32), 1, None, ALU.arith_shift_right
                )
                nc.vector.tensor_sub(ybits[:], magic[:], ybits[:])
                y = ybits[:].bitcast(f32)
                t1 = stats.tile([128, K], f32, tag="t1")
                t2 = stats.tile([128, K], f32, tag="t2")
                for _ in range(1):
                    nc.vector.tensor_mul(t1[:], y, y)
                    nc.vector.tensor_mul(t1[:], t1[:], ssq[:])
                    nc.vector.tensor_scalar(
                        t2[:], t1[:], -0.5, 1.5, ALU.mult, ALU.add
                    )
                    nc.vector.tensor_mul(ybits[:].bitcast(f32), y, t2[:])
                sqs = stats.tile([128, K], f32, tag="sqs")
                nc.vector.tensor_mul(sqs[:], ssq[:], y)  # sqrt(ssq)
                zi2 = stats.tile([128, K], f32, tag="zi2")
                nc.vector.tensor_mul(zi2[:], zi[:], zi[:])
                sn2 = stats.tile([128, K], f32, tag="sn2")
                nc.vector.tensor_mul(sn2[:], ssq[:], zi2[:])
                den = stats.tile([128, K], f32, tag="den")
                nc.vector.tensor_scalar_add(den[:], sn2[:], 0.5)
                rden = stats.tile([128, K], f32, tag="rden")
                nc.vector.reciprocal(out=rden[:], in_=den[:])
                gfac = stats.tile([128, K], f32, tag="gfac")
                nc.vector.tensor_mul(gfac[:], sqs[:], zi2[:])
                nc.vector.tensor_mul(gfac[:], gfac[:], rden[:])

                if not enable_d:
                    continue
                # v in bf16 directly (only the G-matmuls consume it)
                v_bf = work.tile([128, KO], bf16, tag="v_bf")
                for k in range(K):
                    nc.vector.tensor_scalar_mul(
                        v_bf[:, O * k : O * (k + 1)],
                        ssum[:, O * k : O * (k + 1)],
                        gfac[:, k : k + 1],
                    )

                # ---- (d) agreement + b-update + prep of next (b), per wave ----
                st_next = psum_s.tile([128, KO], f32, tag="stilde")
                for w in range(NW):
                    wsl = slice(512 * w, 512 * (w + 1))
                    g_ps = psum_g.tile([128, 512], f32, tag="g_ps")
                    for jj in range(8):
                        g = 8 * w + jj
                        nc.tensor.matmul(
                            g_ps[:, KO * jj : KO * (jj + 1)],
                            lhsT=xn_s[:, 128 * g : 128 * (g + 1)],
                            rhs=v_bf[:],
                            start=True,
                            stop=True,
                        )
                    # pprod straight from PSUM (no scalar copy stage)
                    pprod = work.tile([128, 512], bf16, tag="pprod")
                    nc.vector.tensor_mul(pprod[:], g_ps[:], wt_s[:, wsl])
                    a1w = stats.tile([128, 16], bf16, tag="a1w")
                    with nc.allow_low_precision(
                        reason="agreement partials only feed routing logits"
                    ):
                        nc.vector.tensor_reduce(
                            out=a1w[:].rearrange("p (jj k) -> p jj k", jj=8),
                            in_=pprod[:].rearrange("p (jj k o) -> p jj k o", jj=8, k=K),
                            axis=mybir.AxisListType.X,
                            op=ALU.add,
                        )
                    bsl = b_sb[:, 16 * w : 16 * w + 16]
                    b2_ps = psum_m.tile([128, 16], f32, tag="b2")
                    nc.tensor.matmul(
                        b2_ps[:], lhsT=e88_s[:], rhs=a1w[:], start=True, stop=(it == 0)
                    )
                    if it > 0:
                        nc.tensor.matmul(
                            b2_ps[:], lhsT=id_s[:], rhs=bsl, start=False, stop=True
                        )
                    nc.scalar.activation(out=bsl, in_=b2_ps[:], func=AF.Copy)
                    # exp with o-broadcast done on the scalar engine: e512
                    # [p, (jj,k,o)] = exp(b) bcast over o, so the cwt multiply
                    # below is a plain 2D op
                    e512 = work.tile([128, 512], bf16, tag="e512")
                    e_bc = bass.AP(
                        tensor=bsl.tensor,
                        offset=bsl.offset,
                        ap=[list(bsl.ap[0]), [2, 8], [1, 2], [0, O]],
                    )
                    nc.scalar.activation(
                        out=e512[:].rearrange("p (jj k o) -> p jj k o", jj=8, k=K),
                        in_=e_bc,
                        func=AF.Exp,
                    )
                    nc.gpsimd.tensor_mul(cwt_s[:, wsl], wt_s[:, wsl], e512[:])
                    for jj in range(8):
                        g = 8 * w + jj
                        nc.tensor.matmul(
                            st_next[:],
                            lhsT=xt_s[:, g, :],
                            rhs=cwt_s[:, KO * g : KO * (g + 1)],
                            start=(g == 0),
                            stop=(g == NG - 1),
                        )
                st_ps = st_next

    nc.compile()
    return nc


def _get_program():
    global _PROGRAM
    if _PROGRAM is None:
        import os

        n_iter = int(os.environ.get("KERNEL_N_ITER", str(NUM_ITER)))
        enable_d = os.environ.get("KERNEL_ENABLE_D", "1") == "1"
        _PROGRAM = _build_program(n_iter, enable_d)
    return _PROGRAM


def _prep_inputs(x, W):
    import ml_dtypes

    bf = ml_dtypes.bfloat16
    x = np.asarray(x, dtype=np.float32)
    W = np.asarray(W, dtype=np.float32)
    # 1/B folded in: the e88 matmul accumulates mean-agreement into b directly
    e88 = (
        np.kron(np.eye(8, dtype=np.float32), np.ones((16, 16), np.float32)) / B
    ).astype(bf)
    id128 = np.eye(128, dtype=np.float32).astype(bf)
    in_maps = []
    for c in range(NCORES):
        rs, re = c * RLOC, (c + 1) * RLOC
        xs = x[:, rs:re, :].astype(bf)  # [B, RLOC, I]
        xn = np.ascontiguousarray(xs.reshape(B, RLOC * I))
        # [p=(j,i), (g, b)]: fully contiguous per-partition rows for the DMA
        xt = np.ascontiguousarray(
            xs.reshape(B, NG, 8, I).transpose(2, 3, 1, 0).reshape(128, NG * B)
        )
        Wl = W[0, rs:re].astype(bf)  # [RLOC, K, O, I]
        wt = np.ascontiguousarray(
            Wl.reshape(NG, 8, K, O, I).transpose(1, 4, 0, 2, 3).reshape(128, NG * KO)
        )
        in_maps.append({"xn": xn, "xt": xt, "wt": wt, "e88": e88, "id128": id128})
    return in_maps


def run(x, W, trace=False):
    from concourse import bass_utils

    nc = _get_program()
    in_maps = _prep_inputs(x, W)
    res = bass_utils.run_bass_kernel_spmd(
        nc, in_maps, core_ids=list(range(NCORES)), trace=trace
    )
    # unshard: sum the per-core partial s~ / Z, then the final squash
    parts = [np.asarray(res.results[c]["v_out"], np.float32) for c in range(NCORES)]
    tot = np.sum(parts, axis=0)  # [B, KO+2]
    z = tot[0, KO : KO + 2]  # [K]
    s = tot[:, :KO].reshape(B, K, O) / z[None, :, None]
    sn = (s * s).sum(-1, keepdims=True)
    v = sn * s / ((0.5 + sn) * np.sqrt(sn))
    return v.astype(np.float32), res


def kernel(x, W):
    v, _ = run(x, W, trace=False)
    return v
